# revision 11
# baseline (speedup 1.0000x reference)
"""Distributed 2-layer GCN (PyG GCNConv semantics) on 8 Trainium2 NeuronCores.

Strategy (self-contained; shapes hardcoded for the target problem):
- Nodes are dealt to 8 cores snake-wise by in-degree (balanced edge load);
  within a core, dsts are degree-sorted into 98 blocks of 128 lanes.
- Aggregation A_hat @ t is computed per dst block: per-edge source rows are
  pulled with dma_gather (InstDMAGatherAnt, 128B transfers on a 256B-stride
  bf16 table) into [128 lanes, slots, 64] rectangles, then segment-summed on
  the vector engine. dinv normalization is folded into the table rows
  (t'[n] = dinv[n] * t[n]) so no per-edge scaling is needed; the per-dst
  dinv factor + bias (+ relu) are applied per block.
- int16 gather indices cover 65536 rows per window (signed offsets from a
  base row); sources are 2-colored (balanced per dst) into two windows.
- Three NEFF launches: N1 (t1' = dinv*(x@W1) slices), N2 (aggregate layer 1,
  relu, t2' = dinv*(h1@W2) slices), N3 (aggregate layer 2, output z2).
  The host concatenates slices into the full gather table between launches.
"""
import numpy as np
import ml_dtypes

from concourse import bass, bacc, mybir
import concourse.tile as tile
from concourse.bass_utils import run_bass_kernel_spmd
from concourse.masks import make_identity
from concourse._compat import cdiv, exact_div

BF16 = mybir.dt.bfloat16
F32 = mybir.dt.float32
I16 = mybir.dt.int16

N_NODES = 100000
IN_C, HID_C, OUT_C = 128, 64, 64
NC = 8
PER_CORE = 12544
BLOCKS = 98
WIN_CAP = 65536
CALL_SLOTS = 96
NW = 2
CORE_IDS = list(range(NC))

# ----------------------------------------------------------------------------
# host planner
# ----------------------------------------------------------------------------

def plan_schedule(edge_index, seed=12345):
    rng = np.random.default_rng(seed)
    src = np.ascontiguousarray(edge_index[0]).astype(np.int64)
    dst = np.ascontiguousarray(edge_index[1]).astype(np.int64)

    loops = np.arange(N_NODES, dtype=np.int64)
    e_src = np.concatenate([src, loops])
    e_dst = np.concatenate([dst, loops])

    deg = np.bincount(e_dst, minlength=N_NODES)
    dinv = (1.0 / np.sqrt(deg.astype(np.float64))).astype(np.float32)

    order = np.argsort(-deg, kind="stable")
    pat = np.concatenate([np.arange(NC), np.arange(NC - 1, -1, -1)])
    reps = (N_NODES + 2 * NC - 1) // (2 * NC)
    core_of_rank = np.tile(pat, reps)[:N_NODES]

    node_ids = np.full((NC, PER_CORE), -1, np.int64)
    lane_of_node = np.empty(N_NODES, np.int32)
    block_of_node = np.empty(N_NODES, np.int32)
    core_of_node = np.empty(N_NODES, np.int32)
    for c in range(NC):
        nodes = order[core_of_rank == c]
        node_ids[c, :len(nodes)] = nodes
        idx = np.arange(len(nodes))
        lane_of_node[nodes] = idx % 128
        block_of_node[nodes] = idx // 128
        core_of_node[nodes] = c

    # balanced 2-coloring of sources (per-dst balance, paired flips)
    x = rng.integers(0, 2, N_NODES).astype(np.int8)
    for it in range(30):
        sA = (x[e_src] == 0).astype(np.float64)
        bal = np.bincount(e_dst, weights=2 * sA - 1, minlength=N_NODES)
        delta = np.where(x[e_src] == 0, -2.0, 2.0)
        b = bal[e_dst]
        peg = b * b - (b + delta) ** 2
        gain = np.bincount(e_src, weights=peg, minlength=N_NODES)
        want = (gain > 0) & (rng.random(N_NODES) < (0.5 if it < 20 else 0.25))
        c0 = np.where(want & (x == 0))[0]
        c1 = np.where(want & (x == 1))[0]
        k = min(len(c0), len(c1))
        if k == 0:
            break
        fl = np.concatenate([c0[:k], c1[:k]])
        x[fl] ^= 1
    color = x.astype(np.int64)

    cnt_w = np.bincount(color, minlength=NW)
    assert cnt_w.max() < WIN_CAP - 1
    rank_in_color = np.empty(N_NODES, np.int64)
    for w in range(NW):
        m = color == w
        rank_in_color[m] = np.arange(cnt_w[w])
    pos = color * WIN_CAP + rank_in_color
    win_base = np.array([32768, WIN_CAP + 32768])
    zero_row = np.array([w * WIN_CAP + max(32768, int(cnt_w[w])) for w in range(NW)])
    pad_local = (zero_row - win_base).astype(np.int16)
    t_rows = (int(zero_row.max()) + 1 + 127) // 128 * 128
    local = (pos - win_base[color]).astype(np.int32)

    ej = block_of_node[e_dst].astype(np.int64)
    ep = lane_of_node[e_dst].astype(np.int64)
    ec = core_of_node[e_dst].astype(np.int64)
    ew = color[e_src]

    dw_key = e_dst * NW + ew
    cnt_dw = np.bincount(dw_key, minlength=N_NODES * NW).reshape(N_NODES, NW)
    L = np.zeros((BLOCKS, NW), np.int64)
    for w in range(NW):
        np.maximum.at(L[:, w], block_of_node.astype(np.int64), cnt_dw[:, w])

    # window-region slot layout: per window, blocks' ranges concatenated in J
    # order; the region is chunked into calls of <= PAYLOAD slots, each call
    # gets one trailing all-pad guard slot (keeps the fw's trailing-negative
    # drop from ever firing).
    PAYLOAD = CALL_SLOTS - 1
    R = np.zeros((BLOCKS, NW), np.int64)          # region-relative range start
    region_len = np.zeros(NW, np.int64)
    for w in range(NW):
        R[:, w] = np.cumsum(np.r_[0, L[:-1, w]])
        region_len[w] = R[-1, w] + L[-1, w]

    calls = []                                     # dicts: w, slots, col0
    call_col0_by_wk = {}
    col = 0
    for w in range(NW):
        nchunks = max(1, int(cdiv(int(region_len[w]), PAYLOAD)))
        for k in range(nchunks):
            pay = min(PAYLOAD, int(region_len[w]) - k * PAYLOAD)
            calls.append(dict(w=w, slots=pay + 1, col0=col, k=k))
            call_col0_by_wk[(w, k)] = col
            col += pay + 1
    total_slots = col

    # per-block pieces: (call_index, offset_in_call, length)
    call_index_by_wk = {(c_["w"], c_["k"]): i for i, c_ in enumerate(calls)}
    pieces = [[] for _ in range(BLOCKS)]
    for J in range(BLOCKS):
        for w in range(NW):
            q0, q1 = int(R[J, w]), int(R[J, w] + L[J, w])
            q = q0
            while q < q1:
                k = q // PAYLOAD
                off = q % PAYLOAD
                ln = min(q1 - q, PAYLOAD - off)
                pieces[J].append((call_index_by_wk[(w, k)], off, ln))
                q += ln

    sort_idx = np.lexsort((np.arange(len(e_src)), dw_key))
    sorted_key = dw_key[sort_idx]
    first = np.r_[True, sorted_key[1:] != sorted_key[:-1]]
    grp_start = np.where(first)[0]
    cum = np.arange(len(e_src)) - np.repeat(
        grp_start, np.diff(np.r_[grp_start, len(e_src)]))
    slot_of_edge = np.empty(len(e_src), np.int64)
    slot_of_edge[sort_idx] = cum

    # edge -> global stream column
    q_edge = R[ej, ew] + slot_of_edge              # region-relative slot
    k_edge = q_edge // PAYLOAD
    off_edge = q_edge % PAYLOAD
    wk_col0 = np.zeros((NW, int(max(k_edge.max() + 1, 1))), np.int64)
    for (w, k), c0 in call_col0_by_wk.items():
        wk_col0[w, k] = c0
    slot_abs = wk_col0[ew, k_edge] + off_edge
    stream_pos = slot_abs * 128 + ep

    pad_fill = np.empty(total_slots, np.int16)
    for c_ in calls:
        pad_fill[c_["col0"]:c_["col0"] + c_["slots"]] = pad_local[c_["w"]]
    streams = np.empty((NC, total_slots * 128), np.int16)
    streams[:] = np.repeat(pad_fill, 128)[None, :]
    loc16 = local.astype(np.int16)
    for c in range(NC):
        m = ec == c
        streams[c, stream_pos[m]] = loc16[e_src[m]]

    idx_wrapped = np.empty((NC, 128, total_slots * 8), np.int16)
    for c in range(NC):
        w16 = streams[c].reshape(-1, 16).T
        idx_wrapped[c] = np.tile(w16, (8, 1))

    return dict(
        dinv=dinv, node_ids=node_ids, pos=pos, win_base=win_base,
        pad_local=pad_local, t_rows=t_rows, L=L, calls=calls,
        pieces=pieces, total_slots=total_slots, idx_wrapped=idx_wrapped,
    )


# ----------------------------------------------------------------------------
# device builders
# ----------------------------------------------------------------------------

def _dma_gather_128(eng, out_ap, in_ap, idxs_ap, num_idxs, single_packet=True):
    """dma_gather with elem_size=64 bf16 (128B transfer, 256B stride) —
    replicates bass.dma_gather minus the elem_size%256 assert (verified on HW)."""
    stride_bytes_256 = exact_div(128 * mybir.dt.size(in_ap.dtype), 256)
    _in_ap = eng.lower_ap_dma(in_ap, for_custom_bir_dma=True)
    _idxs_ap = eng.lower_ap(idxs_ap)
    _out_ap = eng.lower_ap(out_ap)
    return eng.add_instruction(
        mybir.InstDMAGatherAnt(
            name=eng.bass.get_next_instruction_name(),
            ins=[*_in_ap, _idxs_ap, eng.lower_val_access(eng.to_reg(num_idxs))],
            outs=[_out_ap],
            transpose=False,
            num_idxs=num_idxs,
            elem_size=64,
            stride_bytes_256=stride_bytes_256,
            gen_mode=0,
            single_packet=single_packet,
            queue_num=0,
            sbuf_tokens_per_rank=0,
            sbuf_free_dim_per_rank=0,
            sbuf_free_dim_pad_per_rank=0,
            sbuf_byte_offset=0,
        )
    )


def build_n1():
    nc = bacc.Bacc()
    xT = nc.declare_dram_parameter("xT", [128, PER_CORE], BF16, isOutput=False)
    W1 = nc.declare_dram_parameter("W1", [IN_C, HID_C], BF16, isOutput=False)
    dinv_sb_d = nc.declare_dram_parameter("dinv_sb", [128, BLOCKS], F32, isOutput=False)
    t1c = nc.declare_dram_parameter("t1c", [PER_CORE, HID_C], BF16, isOutput=True)

    with tile.TileContext(nc) as tc:
        with (
            tc.tile_pool(name="const", bufs=1) as cpool,
            tc.tile_pool(name="sbuf", bufs=3) as sbuf,
            tc.tile_pool(name="psum", bufs=3, space="PSUM") as psum,
        ):
            xT_t = cpool.tile([128, PER_CORE], BF16)
            nc.sync.dma_start(out=xT_t[:], in_=xT[:])
            W1_t = cpool.tile([IN_C, HID_C], BF16)
            nc.sync.dma_start(out=W1_t[:], in_=W1[:])
            dinv_t = cpool.tile([128, BLOCKS], F32)
            nc.sync.dma_start(out=dinv_t[:], in_=dinv_sb_d[:])

            for b in range(BLOCKS):
                pt = psum.tile([128, HID_C], F32, tag="mm")
                nc.tensor.matmul(pt[:], lhsT=xT_t[:, b * 128:(b + 1) * 128],
                                 rhs=W1_t[:], start=True, stop=True)
                ot = sbuf.tile([128, HID_C], BF16, tag="out")
                nc.scalar.activation(out=ot[:], in_=pt[:],
                                     func=mybir.ActivationFunctionType.Copy,
                                     scale=dinv_t[:, b:b + 1])
                nc.sync.dma_start(out=t1c[b * 128:(b + 1) * 128, :], in_=ot[:])
    nc.compile()
    return nc


def build_n23(plan, last):
    """last=False -> N2 (agg layer1 + relu + t2' = dinv*(h1@W2)), outputs bf16.
    last=True  -> N3 (agg layer2 + bias), outputs f32 z2."""
    t_rows = plan["t_rows"]
    total8 = plan["total_slots"] * 8
    calls = plan["calls"]
    pieces = plan["pieces"]
    win_base = plan["win_base"]

    nc = bacc.Bacc()
    table = nc.declare_dram_parameter("table", [t_rows, 128], BF16, isOutput=False)
    idxs = nc.declare_dram_parameter("idxs", [128, total8], I16, isOutput=False)
    dinv_sb_d = nc.declare_dram_parameter("dinv_sb", [128, BLOCKS], F32, isOutput=False)
    brep = nc.declare_dram_parameter("brep", [128, HID_C if not last else OUT_C],
                                     F32, isOutput=False)
    if not last:
        W2 = nc.declare_dram_parameter("W2", [HID_C, OUT_C], BF16, isOutput=False)
        outp = nc.declare_dram_parameter("out", [PER_CORE, OUT_C], BF16, isOutput=True)
    else:
        outp = nc.declare_dram_parameter("out", [PER_CORE, OUT_C], F32, isOutput=True)

    with tile.TileContext(nc) as tc:
        with (
            tc.tile_pool(name="const", bufs=1) as cpool,
            tc.tile_pool(name="msg", bufs=8) as msgp,
            tc.tile_pool(name="work", bufs=4) as work,
            tc.tile_pool(name="psum", bufs=3, space="PSUM") as psum,
        ):
            idx_t = cpool.tile([128, total8], I16)
            nc.sync.dma_start(out=idx_t[:], in_=idxs[:])
            dinv_t = cpool.tile([128, BLOCKS], F32)
            nc.sync.dma_start(out=dinv_t[:], in_=dinv_sb_d[:])
            b_t = cpool.tile([128, HID_C if not last else OUT_C], F32)
            nc.sync.dma_start(out=b_t[:], in_=brep[:])
            if not last:
                W2_t = cpool.tile([HID_C, OUT_C], BF16)
                nc.sync.dma_start(out=W2_t[:], in_=W2[:])
                ident = cpool.tile([128, 128], F32)
                make_identity(nc, ident[:])

            # issue gather calls lazily (in first-use order), process blocks
            mtiles = {}

            def issue(ci):
                c_ = calls[ci]
                slots = c_["slots"]
                mt = msgp.tile([128, slots, 64], BF16, tag="msg")
                base = int(win_base[c_["w"]])
                _dma_gather_128(
                    nc.gpsimd, mt[:], table[base:, :64],
                    idx_t[:, c_["col0"] * 8:(c_["col0"] + slots) * 8],
                    num_idxs=slots * 128, single_packet=False)
                mtiles[ci] = mt
                return mt

            for J in range(BLOCKS):
                acc = work.tile([128, HID_C], F32, tag="acc")
                first_red = True
                for (ci, off, ln) in pieces[J]:
                    mt = mtiles.get(ci) or issue(ci)
                    view = mt[:, off:off + ln, :].rearrange("p l f -> p f l")
                    if first_red:
                        nc.vector.tensor_reduce(
                            out=acc[:], in_=view, axis=mybir.AxisListType.X,
                            op=mybir.AluOpType.add)
                        first_red = False
                    else:
                        tmp = work.tile([128, HID_C], F32, tag="tmp")
                        nc.vector.tensor_reduce(
                            out=tmp[:], in_=view, axis=mybir.AxisListType.X,
                            op=mybir.AluOpType.add)
                        nc.vector.tensor_tensor(
                            out=acc[:], in0=acc[:], in1=tmp[:],
                            op=mybir.AluOpType.add)
                # z = acc * dinv + b
                z = work.tile([128, HID_C], F32, tag="z")
                nc.vector.scalar_tensor_tensor(
                    out=z[:], in0=acc[:], scalar=dinv_t[:, J:J + 1], in1=b_t[:],
                    op0=mybir.AluOpType.mult, op1=mybir.AluOpType.add)
                if last:
                    nc.sync.dma_start(out=outp[J * 128:(J + 1) * 128, :], in_=z[:])
                else:
                    h1 = work.tile([128, HID_C], F32, tag="h1")
                    nc.scalar.activation(out=h1[:], in_=z[:],
                                         func=mybir.ActivationFunctionType.Relu)
                    h1T_p = psum.tile([HID_C, 128], F32, tag="h1T")
                    nc.tensor.transpose(out=h1T_p[:], in_=h1[:],
                                        identity=ident[:])
                    h1T = work.tile([HID_C, 128], BF16, tag="h1Ts")
                    nc.scalar.activation(out=h1T[:], in_=h1T_p[:],
                                         func=mybir.ActivationFunctionType.Copy)
                    t2_p = psum.tile([128, OUT_C], F32, tag="t2")
                    nc.tensor.matmul(t2_p[:], lhsT=h1T[:], rhs=W2_t[:],
                                     start=True, stop=True)
                    t2 = work.tile([128, OUT_C], BF16, tag="t2s")
                    nc.scalar.activation(out=t2[:], in_=t2_p[:],
                                         func=mybir.ActivationFunctionType.Copy,
                                         scale=dinv_t[:, J:J + 1])
                    nc.sync.dma_start(out=outp[J * 128:(J + 1) * 128, :],
                                      in_=t2[:])
    nc.compile()
    return nc


# ----------------------------------------------------------------------------
# top-level kernel
# ----------------------------------------------------------------------------

_CACHE = {}
LAST_EXEC_NS = []     # filled per launch when GCN_TRACE=1
LAST_TRACE_DIRS = []
TRACE = False


def _ensure_ntff_hook():
    """This image lacks antenv.axon_hooks; synthesize it and register the
    ctypes NTFF hook from trn_agent_boot so trace=True yields exec_time_ns."""
    import sys, types
    try:
        from antenv.axon_hooks import get_axon_ntff_profile_hook  # noqa
        return
    except ImportError:
        pass
    mod = types.ModuleType("antenv.axon_hooks")
    _state = {"hook": None}
    mod.set_axon_ntff_profile_hook = lambda h: _state.__setitem__("hook", h)
    mod.get_axon_ntff_profile_hook = lambda: _state["hook"]
    sys.modules["antenv.axon_hooks"] = mod
    import antenv
    antenv.axon_hooks = mod
    try:
        from trn_agent_boot.trn_boot import _ntff_profile_via_ctypes
        mod.set_axon_ntff_profile_hook(
            _ntff_profile_via_ctypes("/opt/axon/libaxon_pjrt.so"))
    except Exception:
        pass


def _run(nc, in_maps, trace):
    import tempfile
    kw = {}
    if trace:
        _ensure_ntff_hook()
        kw["tmpdir"] = tempfile.mkdtemp(prefix="gcn_trace_")
    r = run_bass_kernel_spmd(nc, in_maps, core_ids=CORE_IDS, trace=trace, **kw)
    if trace:
        LAST_EXEC_NS.append(r.exec_time_ns)
        LAST_TRACE_DIRS.append(kw.get("tmpdir"))
    return r.results


def kernel(x, edge_index, W1, b1, W2, b2):
    import os
    trace = TRACE or bool(os.environ.get("GCN_TRACE"))
    LAST_EXEC_NS.clear()

    x = np.asarray(x)
    edge_index = np.asarray(edge_index)
    W1 = np.asarray(W1, dtype=np.float32)
    b1 = np.asarray(b1, dtype=np.float32)
    W2 = np.asarray(W2, dtype=np.float32)
    b2 = np.asarray(b2, dtype=np.float32)

    key = hash(edge_index.tobytes())
    if key in _CACHE:
        plan, ncs = _CACHE[key]
    else:
        plan = plan_schedule(edge_index)
        ncs = {}
        _CACHE[key] = (plan, ncs)

    dinv = plan["dinv"]
    node_ids = plan["node_ids"]

    # per-core host arrays
    x32 = x.astype(np.float32)
    xT_in, dinv_in = [], []
    for c in range(NC):
        ids = node_ids[c]
        m = ids >= 0
        xs = np.zeros((PER_CORE, IN_C), np.float32)
        xs[m] = x32[ids[m]]
        xT_in.append(np.ascontiguousarray(xs.T).astype(ml_dtypes.bfloat16))
        dv = np.zeros(PER_CORE, np.float32)
        dv[m] = dinv[ids[m]]
        dinv_in.append(np.ascontiguousarray(dv.reshape(BLOCKS, 128).T))

    W1b = W1.astype(ml_dtypes.bfloat16)
    W2b = W2.astype(ml_dtypes.bfloat16)
    b1rep = np.tile(b1[None, :], (128, 1)).astype(np.float32)
    b2rep = np.tile(b2[None, :], (128, 1)).astype(np.float32)

    # --- N1: t1' slices ---
    if "n1" not in ncs:
        ncs["n1"] = build_n1()
    maps1 = [{"xT": xT_in[c], "W1": W1b, "dinv_sb": dinv_in[c]} for c in range(NC)]
    r1 = _run(ncs["n1"], maps1, trace)

    def assemble_table(slices):
        tab = np.zeros((plan["t_rows"], 128), ml_dtypes.bfloat16)
        for c in range(NC):
            ids = node_ids[c]
            m = ids >= 0
            tab[plan["pos"][ids[m]], :64] = slices[c][m]
        return tab

    table1 = assemble_table([r1[c]["t1c"] for c in range(NC)])

    # --- N2: aggregate layer 1, produce t2' slices ---
    if "n2" not in ncs:
        ncs["n2"] = build_n23(plan, last=False)
    maps2 = [{"table": table1, "idxs": plan["idx_wrapped"][c],
              "dinv_sb": dinv_in[c], "brep": b1rep, "W2": W2b}
             for c in range(NC)]
    r2 = _run(ncs["n2"], maps2, trace)

    table2 = assemble_table([r2[c]["out"] for c in range(NC)])

    # --- N3: aggregate layer 2 ---
    if "n3" not in ncs:
        ncs["n3"] = build_n23(plan, last=True)
    maps3 = [{"table": table2, "idxs": plan["idx_wrapped"][c],
              "dinv_sb": dinv_in[c], "brep": b2rep}
             for c in range(NC)]
    r3 = _run(ncs["n3"], maps3, trace)

    out = np.zeros((N_NODES, OUT_C), np.float32)
    for c in range(NC):
        ids = node_ids[c]
        m = ids >= 0
        out[ids[m]] = r3[c]["out"][m]
    return out
